# revision 4
# baseline (speedup 1.0000x reference)
"""GAT message-passing kernel for Trainium2 (Bass/Tile), 8-core data parallel.

Problem: nn_GAT1 — per batch b:
    h = x @ W_pre                                   [N, U]
    e_s = h @ a_snd ; e_r = h @ a_rec               [N]
    logits[s, r] = leaky_relu(e_s[s] + e_r[r], 0.2)
    att = softmax over senders s (edges only, adj + self-loops)
    out[s, u] = sum_r att[s, r] * h[r, u]

Sharding: data-parallel over batch (B=8 -> one batch per NeuronCore).

Device layout: everything receiver-major ("transposed", [r on partitions, s free]):
    adjT[r, s] = adj[s, r]   (host provides the transposed view; device casts
                              fp32->bf16 during the DMA itself via SWDGE)
    logitsT[r, s] = e_s[s] (free-dim vector, broadcast matrix E)
                  + e_r[r] (per-partition activation bias)
    pmT = exp(prelu(logitsT)) * adjT    per-tile, with the softmax denominator
          produced for free by scalar_tensor_tensor's accum_out
    outT[u, s] = sum_r (h[r, u] * inv_den[r]) * pmT[r, s]   (PE, weight-stationary)
Host transposes outT back when gathering.
"""
import os
import sys

sys.path.insert(0, "/opt/trn_rl_repo")
sys.path.insert(0, "/opt/trn_rl_repo/concourse")

import numpy as np

import concourse.bacc as bacc
import concourse.tile as tile
from concourse import mybir
from concourse.bass_utils import run_bass_kernel_spmd

B, N, F, U = 8, 2048, 128, 128
P = 128
NT = N // P          # 16 row tiles
ALPHA = 0.2          # leaky-relu slope

# number of r-tiles whose leaky-relu runs on the Scalar engine (rest on Vector)
K_ACT = int(os.environ.get("GAT_K_ACT", "5"))
DMA_CHUNK = int(os.environ.get("GAT_DMA_CHUNK", "4"))   # r-tiles per adjT DMA

f32 = mybir.dt.float32
bf16 = mybir.dt.bfloat16
AF = mybir.ActivationFunctionType
OP = mybir.AluOpType

_cache = {}


def _build_nc():
    nc = bacc.Bacc("TRN2", target_bir_lowering=False, debug=False,
                   enable_asserts=False, num_devices=B)

    x_d = nc.dram_tensor("x", [N, F], f32, kind="ExternalInput").ap()
    adjt_d = nc.dram_tensor("adjt", [N, N], f32, kind="ExternalInput").ap()
    w_d = nc.dram_tensor("w", [F, U], f32, kind="ExternalInput").ap()
    asnd_d = nc.dram_tensor("asnd", [U, 1], f32, kind="ExternalInput").ap()
    arec_d = nc.dram_tensor("arec", [U, 1], f32, kind="ExternalInput").ap()
    eye_d = nc.dram_tensor("eye", [P, P], f32, kind="ExternalInput").ap()
    outT_d = nc.dram_tensor("outT", [U, N], f32, kind="ExternalOutput").ap()

    with tile.TileContext(nc) as tc:
        with (
            tc.tile_pool(name="const", bufs=1) as const,
            tc.tile_pool(name="setup", bufs=2) as setup,
            tc.tile_pool(name="spsum", bufs=2, space="PSUM") as spsum,
            tc.tile_pool(name="adjp", bufs=3) as adjp,
            tc.tile_pool(name="work", bufs=3) as work,
            tc.tile_pool(name="mpsum", bufs=1, space="PSUM") as mpsum,
        ):
            # ---------------- constants ----------------
            w_sb = const.tile([F, U], f32)
            nc.sync.dma_start(out=w_sb[:], in_=w_d)
            asnd_sb = const.tile([U, 1], f32)
            nc.sync.dma_start(out=asnd_sb[:], in_=asnd_d)
            arec_sb = const.tile([U, 1], f32)
            nc.sync.dma_start(out=arec_sb[:], in_=arec_d)
            eye_sb = const.tile([P, P], f32)
            nc.sync.dma_start(out=eye_sb[:], in_=eye_d)
            ones_bf = const.tile([1, P], bf16)
            nc.vector.memset(ones_bf[:], 1.0)

            # ---------------- x load + xT ----------------
            x_sb = const.tile([P, NT, F], f32)
            nc.sync.dma_start(out=x_sb[:], in_=x_d.rearrange("(t p) f -> p t f", p=P))

            xT_sb = const.tile([F, NT, P], f32)   # xT tile t = x tile t transposed
            for g in range(4):
                ps = spsum.tile([P, 512], f32, tag="tp")
                for k in range(4):
                    i = 4 * g + k
                    nc.tensor.transpose(ps[:, k * P:(k + 1) * P], x_sb[:, i, :], eye_sb[:])
                for k in range(4):
                    i = 4 * g + k
                    nc.any.tensor_copy(xT_sb[:, i, :], ps[:, k * P:(k + 1) * P])

            # ---------------- W^T, w_s, w_r ----------------
            psw = spsum.tile([P, P], f32, tag="small")
            nc.tensor.transpose(psw[:], w_sb[:], eye_sb[:])
            wT_sb = setup.tile([U, F], f32)
            nc.any.tensor_copy(wT_sb[:], psw[:])

            ps_wsr = spsum.tile([P, 2], f32, tag="small")
            nc.tensor.matmul(ps_wsr[:, 0:1], lhsT=wT_sb[:], rhs=asnd_sb[:],
                             start=True, stop=True)
            nc.tensor.matmul(ps_wsr[:, 1:2], lhsT=wT_sb[:], rhs=arec_sb[:],
                             start=True, stop=True)
            wsr_sb = setup.tile([F, 2], f32)
            nc.any.tensor_copy(wsr_sb[:], ps_wsr[:])

            # ---------------- e_r columns (one per r-tile) ----------------
            ps_er = spsum.tile([P, NT], f32, tag="small")
            for j in range(NT):
                nc.tensor.matmul(ps_er[:, j:j + 1], lhsT=xT_sb[:, j, :],
                                 rhs=wsr_sb[:, 1:2], start=True, stop=True)
            er_sb = const.tile([P, NT], f32)
            nc.any.tensor_copy(er_sb[:], ps_er[:])
            er02_sb = const.tile([P, NT], f32)
            nc.vector.tensor_scalar(er02_sb[:], er_sb[:], ALPHA, None, op0=OP.mult)

            # ---------------- e_s row + broadcast matrix E ----------------
            es_row = setup.tile([1, N], bf16)
            for c in range(4):
                ps_es = spsum.tile([1, 512], f32, tag="small")
                nc.tensor.matmul(ps_es[:], lhsT=wsr_sb[:, 0:1],
                                 rhs=xT_sb.rearrange("f t p -> f (t p)")[:, c * 512:(c + 1) * 512],
                                 start=True, stop=True)
                nc.any.tensor_copy(es_row[:, c * 512:(c + 1) * 512], ps_es[:])

            E_sb = const.tile([P, N], bf16)
            for c in range(4):
                ps_E = spsum.tile([P, 512], f32, tag="tp")
                nc.tensor.matmul(ps_E[:], lhsT=ones_bf[:],
                                 rhs=es_row[:, c * 512:(c + 1) * 512],
                                 start=True, stop=True)
                nc.any.tensor_copy(E_sb[:, c * 512:(c + 1) * 512], ps_E[:])

            # ---------------- h (bf16, natural layout) ----------------
            h_sb = const.tile([P, NT, U], bf16)
            for g in range(4):
                psh = spsum.tile([P, 512], f32, tag="tp")
                for k in range(4):
                    i = 4 * g + k
                    nc.tensor.matmul(psh[:, k * P:(k + 1) * P], lhsT=xT_sb[:, i, :],
                                     rhs=w_sb[:], start=True, stop=True)
                for k in range(4):
                    i = 4 * g + k
                    nc.any.tensor_copy(h_sb[:, i, :], psh[:, k * P:(k + 1) * P])

            # ---------------- main loop over r-tiles ----------------
            outT_ps = mpsum.tile([U, N], f32)   # 4 PSUM banks, accumulated over j
            n_chunks = NT // DMA_CHUNK
            for g in range(n_chunks):
                adjt_sb = adjp.tile([P, DMA_CHUNK, N], bf16, tag="adjt")
                nc.gpsimd.dma_start(
                    out=adjt_sb[:],
                    in_=adjt_d[g * DMA_CHUNK * P:(g + 1) * DMA_CHUNK * P, :]
                    .rearrange("(c p) s -> p c s", p=P))
                for cc in range(DMA_CHUNK):
                    j = g * DMA_CHUNK + cc
                    a_j = work.tile([P, N], bf16, tag="a")
                    if j < K_ACT:
                        nc.scalar.activation(a_j[:], E_sb[:], AF.Prelu,
                                             bias=er_sb[:, j:j + 1], scale=1.0,
                                             alpha=ALPHA)
                    else:
                        t_j = work.tile([P, N], bf16, tag="t")
                        nc.vector.tensor_scalar(t_j[:], E_sb[:], ALPHA,
                                                er02_sb[:, j:j + 1],
                                                op0=OP.mult, op1=OP.add)
                        nc.vector.scalar_tensor_tensor(
                            a_j[:], E_sb[:], er_sb[:, j:j + 1], t_j[:],
                            op0=OP.add, op1=OP.max)
                    p_j = work.tile([P, N], bf16, tag="p")
                    nc.scalar.activation(p_j[:], a_j[:], AF.Exp)
                    pm_j = work.tile([P, N], bf16, tag="pm")
                    den_j = work.tile([P, 1], f32, tag="den")
                    nc.vector.scalar_tensor_tensor(
                        pm_j[:], p_j[:], 0.0, adjt_sb[:, cc, :],
                        op0=OP.bypass, op1=OP.mult, accum_out=den_j[:])
                    inv_j = work.tile([P, 1], f32, tag="inv")
                    nc.vector.reciprocal(inv_j[:], den_j[:])
                    hp_j = work.tile([P, U], bf16, tag="hp")
                    nc.vector.tensor_scalar(hp_j[:], h_sb[:, j, :], inv_j[:], None,
                                            op0=OP.mult)
                    for c in range(4):
                        nc.tensor.matmul(outT_ps[:, c * 512:(c + 1) * 512],
                                         lhsT=hp_j[:],
                                         rhs=pm_j[:, c * 512:(c + 1) * 512],
                                         start=(j == 0), stop=(j == NT - 1))

            # ---------------- store ----------------
            outT_sb = setup.tile([U, N], f32)
            for c in range(4):
                nc.any.tensor_copy(outT_sb[:, c * 512:(c + 1) * 512],
                                   outT_ps[:, c * 512:(c + 1) * 512])
            nc.sync.dma_start(out=outT_d, in_=outT_sb[:])

    nc.compile()
    return nc


def kernel(x, adj, W_pre, a_snd, a_rec):
    """Full inputs in, full output out. Shards batch across 8 NeuronCores."""
    if "nc" not in _cache:
        _cache["nc"] = _build_nc()
    nc = _cache["nc"]

    x = np.ascontiguousarray(np.asarray(x, dtype=np.float32))
    adj = np.asarray(adj, dtype=np.float32)
    W_pre = np.ascontiguousarray(np.asarray(W_pre, dtype=np.float32))
    a_snd = np.ascontiguousarray(np.asarray(a_snd, dtype=np.float32).reshape(U, 1))
    a_rec = np.ascontiguousarray(np.asarray(a_rec, dtype=np.float32).reshape(U, 1))

    # self-loops (reference: min(1, adj + I)), then receiver-major layout per batch
    adjt = np.ascontiguousarray(adj.transpose(0, 2, 1))
    idx = np.arange(N)
    adjt[:, idx, idx] = 1.0

    eye = np.eye(P, dtype=np.float32)
    in_maps = [
        {"x": x[b], "adjt": adjt[b], "w": W_pre, "asnd": a_snd, "arec": a_rec,
         "eye": eye}
        for b in range(B)
    ]
    trace = bool(int(os.environ.get("GAT_TRACE", "0")))
    res = run_bass_kernel_spmd(nc, in_maps, core_ids=list(range(B)), trace=trace,
                               trace_cores=list(range(B)) if trace else None)
    _cache["last_result"] = res
    out = np.stack([np.ascontiguousarray(r["outT"].T) for r in res.results])
    return out.astype(np.float32)


# revision 14
# speedup vs baseline: 1.2556x; 1.2556x over previous
"""GAT message-passing kernel for Trainium2 (Bass/Tile), 8-core data parallel.

Problem: nn_GAT1 — per batch b:
    h = x @ W_pre                                   [N, U]
    e_s = h @ a_snd ; e_r = h @ a_rec               [N]
    logits[s, r] = leaky_relu(e_s[s] + e_r[r], 0.2)
    att = softmax over senders s (edges only, adj + self-loops)
    out[s, u] = sum_r att[s, r] * h[r, u]

Sharding: data-parallel over batch (B=8 -> one batch per NeuronCore).

Device layout is receiver-major ("transposed", r on partitions, s on free):
    adjT[r, s] = adj[s, r]; host provides this view, device casts fp32->bf16
    during the DMA itself (SWDGE cast).
    logitsT[r, s] = e_s[s] (free-axis broadcast matrix E) + e_r[r] (per-
    partition scalar).
    pmT = exp(lrelu(logitsT)) * adjT; softmax denominator = row-sum of pmT
    (free-dim reduce via tensor_scalar accum_out on GpSimd).
    outT[u, s] = sum_r (h[r, u] / den[r]) * pmT[r, s]  (PE, weight-stationary)
Host transposes outT back when gathering.
"""
import os
import sys

sys.path.insert(0, "/opt/trn_rl_repo")
sys.path.insert(0, "/opt/trn_rl_repo/concourse")

import numpy as np

import concourse.bass as bass
import concourse.bacc as bacc
import concourse.tile as tile
from concourse import mybir
from concourse.bass_utils import run_bass_kernel_spmd

B, N, F, U = 8, 2048, 128, 128
P = 128
NT = N // P          # 16 row tiles
ALPHA = 0.2          # leaky-relu slope

# r-tiles whose leaky-relu runs on the Scalar engine (rest on Vector)
K_ACT = int(os.environ.get("GAT_K_ACT", "7"))
DMA_CHUNK = int(os.environ.get("GAT_DMA_CHUNK", "4"))   # r-tiles per adjT DMA
DEN_MODE = os.environ.get("GAT_DEN", "ts")              # "ttr" | "ts"
# ("ttr" = tensor_tensor_reduce: crashes the device on this runtime — keep "ts")

f32 = mybir.dt.float32
f32r = mybir.dt.float32r
bf16 = mybir.dt.bfloat16
AF = mybir.ActivationFunctionType
OP = mybir.AluOpType

_cache = {}


def _build_nc():
    nc = bacc.Bacc("TRN2", target_bir_lowering=False, debug=False,
                   enable_asserts=False, num_devices=B)

    x_d = nc.dram_tensor("x", [N, F], f32, kind="ExternalInput").ap()
    adjt_d = nc.dram_tensor("adjt", [N, N], f32, kind="ExternalInput").ap()
    w_d = nc.dram_tensor("w", [F, U], f32, kind="ExternalInput").ap()
    asnd_d = nc.dram_tensor("asnd", [U, 1], f32, kind="ExternalInput").ap()
    arec_d = nc.dram_tensor("arec", [U, 1], f32, kind="ExternalInput").ap()
    eye_d = nc.dram_tensor("eye", [P, P], f32, kind="ExternalInput").ap()
    outT_d = nc.dram_tensor("outT", [U, N], f32, kind="ExternalOutput").ap()

    with tile.TileContext(nc) as tc:
        with (
            tc.tile_pool(name="const", bufs=1) as const,
            tc.tile_pool(name="setup", bufs=2) as setup,
            tc.tile_pool(name="spsum", bufs=2, space="PSUM") as spsum,
            tc.tile_pool(name="adjp", bufs=3) as adjp,
            tc.tile_pool(name="work", bufs=3) as work,
            tc.tile_pool(name="mpsum", bufs=1, space="PSUM") as mpsum,
        ):
            # ---------------- constants ----------------
            w_sb = const.tile([F, U], f32)
            nc.sync.dma_start(out=w_sb[:], in_=w_d)
            asnd_f = const.tile([U, 1], f32)
            nc.sync.dma_start(out=asnd_f[:], in_=asnd_d)
            arec_f = const.tile([U, 1], f32)
            nc.sync.dma_start(out=arec_f[:], in_=arec_d)

            eye_sb = const.tile([P, P], f32)
            nc.sync.dma_start(out=eye_sb[:], in_=eye_d)
            ones_bf = const.tile([1, P], bf16)
            nc.vector.memset(ones_bf[:], 1.0)
            w_bf = const.tile([F, U], bf16)
            nc.vector.tensor_copy(w_bf[:], w_sb[:])

            # ---------------- x load + xT ----------------
            x_sb = const.tile([P, NT, F], f32)
            x_dma = nc.sync.dma_start(out=x_sb[:],
                                      in_=x_d.rearrange("(t p) f -> p t f", p=P))

            xT_sb = const.tile([F, NT, P], f32r)
            xT_bf = const.tile([F, NT, P], bf16)      # bf16 copy for the h matmul
            for g in range(4):
                ps = spsum.tile([P, 512], f32, tag="tp")
                for k in range(4):
                    i = 4 * g + k
                    nc.tensor.transpose(ps[:, k * P:(k + 1) * P], x_sb[:, i, :], eye_sb[:])
                for k in range(4):
                    i = 4 * g + k
                    nc.scalar.copy(xT_sb[:, i, :], ps[:, k * P:(k + 1) * P])
                    nc.vector.tensor_copy(xT_bf[:, i, :], ps[:, k * P:(k + 1) * P])

            # ---------------- W^T, w_s, w_r ----------------
            psw = spsum.tile([P, P], f32, tag="small")
            nc.tensor.transpose(psw[:], w_sb[:], eye_sb[:])
            wT_sb = setup.tile([U, F], f32)
            nc.any.tensor_copy(wT_sb[:], psw[:])

            ps_wsr = spsum.tile([P, 2], f32, tag="small")
            nc.tensor.matmul(ps_wsr[:, 0:1], lhsT=wT_sb[:], rhs=asnd_f[:],
                             start=True, stop=True)
            nc.tensor.matmul(ps_wsr[:, 1:2], lhsT=wT_sb[:], rhs=arec_f[:],
                             start=True, stop=True)
            wsr_sb = setup.tile([F, 2], f32r)
            nc.any.tensor_copy(wsr_sb[:], ps_wsr[:])

            # ---------------- e_s / e_r rows (f32r matmuls, N=512 chunks) -----
            es_row = setup.tile([1, N], bf16)
            er_row = setup.tile([1, N], f32)
            for c in range(4):
                xchunk = xT_sb.rearrange("f t p -> f (t p)")[:, c * 512:(c + 1) * 512]
                ps_es = spsum.tile([1, 512], f32, tag="small")
                nc.tensor.matmul(ps_es[:], lhsT=wsr_sb[:, 0:1], rhs=xchunk,
                                 start=True, stop=True)
                nc.any.tensor_copy(es_row[:, c * 512:(c + 1) * 512], ps_es[:])
                ps_err = spsum.tile([1, 512], f32, tag="small")
                nc.tensor.matmul(ps_err[:], lhsT=wsr_sb[:, 1:2], rhs=xchunk,
                                 start=True, stop=True)
                nc.any.tensor_copy(er_row[:, c * 512:(c + 1) * 512], ps_err[:])

            # e_r as per-partition columns: 16 tiny [1,128]->[128,1] transposes
            ps_er = spsum.tile([P, NT], f32, tag="small")
            for j in range(NT):
                nc.tensor.transpose(ps_er[:, j:j + 1],
                                    er_row[:, j * P:(j + 1) * P],
                                    eye_sb[0:1, 0:1])
            er_sb = const.tile([P, NT], f32)
            nc.any.tensor_copy(er_sb[:], ps_er[:])
            er02_sb = const.tile([P, NT], f32)
            nc.vector.tensor_scalar(er02_sb[:], er_sb[:], ALPHA, None, op0=OP.mult)

            E_sb = const.tile([P, N], bf16)
            for c in range(4):
                ps_E = spsum.tile([P, 512], f32, tag="tp")
                nc.tensor.matmul(ps_E[:], lhsT=ones_bf[:],
                                 rhs=es_row[:, c * 512:(c + 1) * 512],
                                 start=True, stop=True)
                nc.any.tensor_copy(E_sb[:, c * 512:(c + 1) * 512], ps_E[:])

            # ---------------- h (bf16, natural layout) ----------------
            h_sb = const.tile([P, NT, U], bf16)
            for g in range(4):
                psh = spsum.tile([P, 512], f32, tag="tp")
                for k in range(4):
                    i = 4 * g + k
                    nc.tensor.matmul(psh[:, k * P:(k + 1) * P], lhsT=xT_bf[:, i, :],
                                     rhs=w_bf[:], start=True, stop=True)
                for k in range(4):
                    i = 4 * g + k
                    nc.any.tensor_copy(h_sb[:, i, :], psh[:, k * P:(k + 1) * P])

            # ---------------- main loop over r-tiles ----------------
            outT_ps = mpsum.tile([U, N], f32)   # 4 PSUM banks, accumulated over j
            n_chunks = NT // DMA_CHUNK
            for g in range(n_chunks):
                adjt_sb = adjp.tile([P, DMA_CHUNK, N], bf16, tag="adjt")
                adj_dma = nc.gpsimd.dma_start(
                    out=adjt_sb[:],
                    in_=adjt_d[g * DMA_CHUNK * P:(g + 1) * DMA_CHUNK * P, :]
                    .rearrange("(c p) s -> p c s", p=P))
                if g == 0:
                    # keep the small setup DMAs ahead of the big adjT stream
                    bass._add_dep_helper(adj_dma.ins, x_dma.ins, sync=True,
                                         reason="x before adjT flood")
                for cc in range(DMA_CHUNK):
                    j = g * DMA_CHUNK + cc
                    a_j = work.tile([P, N], bf16, tag="a")
                    if j % 2 == 0 and j // 2 < K_ACT:
                        nc.scalar.activation(a_j[:], E_sb[:], AF.Prelu,
                                             bias=er_sb[:, j:j + 1], scale=1.0,
                                             alpha=ALPHA)
                    else:
                        z_j = work.tile([P, N], bf16, tag="z")
                        nc.vector.tensor_scalar(z_j[:], E_sb[:], er_sb[:, j:j + 1],
                                                None, op0=OP.add)
                        t_j = work.tile([P, N], bf16, tag="t")
                        nc.vector.tensor_scalar(t_j[:], E_sb[:], ALPHA,
                                                er02_sb[:, j:j + 1],
                                                op0=OP.mult, op1=OP.add)
                        nc.vector.tensor_max(a_j[:], z_j[:], t_j[:])
                    p_j = work.tile([P, N], bf16, tag="p")
                    nc.scalar.activation(p_j[:], a_j[:], AF.Exp)
                    pm_j = work.tile([P, N], bf16, tag="pm")
                    den_j = work.tile([P, 1], f32, tag="den")
                    if DEN_MODE == "ttr":
                        nc.vector.tensor_tensor_reduce(
                            pm_j[:], p_j[:], adjt_sb[:, cc, :], 1.0, 0.0,
                            op0=OP.mult, op1=OP.add, accum_out=den_j[:])
                    else:
                        nc.vector.tensor_mul(pm_j[:], p_j[:], adjt_sb[:, cc, :])
                        pm_scr = work.tile([P, N], bf16, tag="pmscr")
                        nc.vector.tensor_scalar(pm_scr[:], pm_j[:], 1.0, 0.0,
                                                op0=OP.mult, op1=OP.add,
                                                accum_out=den_j[:])
                    inv_j = work.tile([P, 1], f32, tag="inv")
                    nc.vector.reciprocal(inv_j[:], den_j[:])
                    hp_j = work.tile([P, U], bf16, tag="hp")
                    nc.vector.tensor_scalar(hp_j[:], h_sb[:, j, :], inv_j[:], None,
                                            op0=OP.mult)
                    for c in range(4):
                        nc.tensor.matmul(outT_ps[:, c * 512:(c + 1) * 512],
                                         lhsT=hp_j[:],
                                         rhs=pm_j[:, c * 512:(c + 1) * 512],
                                         start=(j == 0), stop=(j == NT - 1))

            # ---------------- store ----------------
            outT_sb = setup.tile([U, N], f32)
            for c in range(4):
                nc.any.tensor_copy(outT_sb[:, c * 512:(c + 1) * 512],
                                   outT_ps[:, c * 512:(c + 1) * 512])
            nc.sync.dma_start(out=outT_d, in_=outT_sb[:])

    nc.compile()
    return nc


def kernel(x, adj, W_pre, a_snd, a_rec):
    """Full inputs in, full output out. Shards batch across 8 NeuronCores."""
    if "nc" not in _cache:
        _cache["nc"] = _build_nc()
    nc = _cache["nc"]

    x = np.ascontiguousarray(np.asarray(x, dtype=np.float32))
    adj = np.asarray(adj, dtype=np.float32)
    W_pre = np.ascontiguousarray(np.asarray(W_pre, dtype=np.float32))
    a_snd = np.ascontiguousarray(np.asarray(a_snd, dtype=np.float32).reshape(U, 1))
    a_rec = np.ascontiguousarray(np.asarray(a_rec, dtype=np.float32).reshape(U, 1))

    # self-loops (reference: min(1, adj + I)), then receiver-major layout per batch
    adjt = np.ascontiguousarray(adj.transpose(0, 2, 1))
    idx = np.arange(N)
    adjt[:, idx, idx] = 1.0

    eye = np.eye(P, dtype=np.float32)
    in_maps = [
        {"x": x[b], "adjt": adjt[b], "w": W_pre, "asnd": a_snd, "arec": a_rec,
         "eye": eye}
        for b in range(B)
    ]
    trace = bool(int(os.environ.get("GAT_TRACE", "0")))
    res = run_bass_kernel_spmd(nc, in_maps, core_ids=list(range(B)), trace=trace,
                               trace_cores=list(range(B)) if trace else None)
    _cache["last_result"] = res
    out = np.stack([np.ascontiguousarray(r["outT"].T) for r in res.results])
    return out.astype(np.float32)


# revision 16
# speedup vs baseline: 1.5665x; 1.2476x over previous
"""GAT message-passing kernel for Trainium2 (Bass/Tile), 8-core data parallel.

Problem: nn_GAT1 — per batch b:
    h = x @ W_pre                                   [N, U]
    e_s = h @ a_snd ; e_r = h @ a_rec               [N]
    logits[s, r] = leaky_relu(e_s[s] + e_r[r], 0.2)
    att = softmax over senders s (edges only, adj + self-loops)
    out[s, u] = sum_r att[s, r] * h[r, u]

Sharding: data-parallel over batch (B=8 -> one batch per NeuronCore).

Device layout is receiver-major ("transposed", r on partitions, s on free):
    adjT[r, s] = adj[s, r]; host provides this view, device casts fp32->bf16
    during the DMA itself (SWDGE cast).
    logitsT[r, s] = e_s[s] (free-axis broadcast matrix E) + e_r[r] (per-
    partition scalar).
    pmT = exp(lrelu(logitsT)) * adjT; softmax denominator = row-sum of pmT
    (free-dim reduce via tensor_scalar accum_out on GpSimd).
    outT[u, s] = sum_r (h[r, u] / den[r]) * pmT[r, s]  (PE, weight-stationary)
Host transposes outT back when gathering.
"""
import os
import sys

sys.path.insert(0, "/opt/trn_rl_repo")
sys.path.insert(0, "/opt/trn_rl_repo/concourse")

import numpy as np

import concourse.bass as bass
import concourse.bacc as bacc
import concourse.tile as tile
from concourse import mybir
from concourse.bass_utils import run_bass_kernel_spmd

B, N, F, U = 8, 2048, 128, 128
P = 128
NT = N // P          # 16 row tiles
ALPHA = 0.2          # leaky-relu slope

# r-tiles whose leaky-relu runs on the Scalar engine (rest on Vector)
K_ACT = int(os.environ.get("GAT_K_ACT", "7"))
DMA_CHUNK = int(os.environ.get("GAT_DMA_CHUNK", "4"))   # r-tiles per adjT DMA
Z_ENG = os.environ.get("GAT_Z_ENG", "vector")           # "vector" | "gpsimd"

f32 = mybir.dt.float32
f32r = mybir.dt.float32r
bf16 = mybir.dt.bfloat16
AF = mybir.ActivationFunctionType
OP = mybir.AluOpType

_cache = {}


def _build_nc():
    nc = bacc.Bacc("TRN2", target_bir_lowering=False, debug=False,
                   enable_asserts=False, num_devices=B)

    x_d = nc.dram_tensor("x", [N, F], f32, kind="ExternalInput").ap()
    adjt_d = nc.dram_tensor("adjt", [N, N], f32, kind="ExternalInput").ap()
    w_d = nc.dram_tensor("w", [F, U], f32, kind="ExternalInput").ap()
    asnd_d = nc.dram_tensor("asnd", [U, 1], f32, kind="ExternalInput").ap()
    arec_d = nc.dram_tensor("arec", [U, 1], f32, kind="ExternalInput").ap()
    eye_d = nc.dram_tensor("eye", [P, P], f32, kind="ExternalInput").ap()
    outT_d = nc.dram_tensor("outT", [U, N], f32, kind="ExternalOutput").ap()

    with tile.TileContext(nc) as tc:
        with (
            tc.tile_pool(name="const", bufs=1) as const,
            tc.tile_pool(name="setup", bufs=2) as setup,
            tc.tile_pool(name="spsum", bufs=2, space="PSUM") as spsum,
            tc.tile_pool(name="adjp", bufs=3) as adjp,
            tc.tile_pool(name="work", bufs=3) as work,
            tc.tile_pool(name="mpsum", bufs=1, space="PSUM") as mpsum,
        ):
            # ---------------- constants ----------------
            w_sb = const.tile([F, U], f32)
            nc.sync.dma_start(out=w_sb[:], in_=w_d)
            asnd_f = const.tile([U, 1], f32)
            nc.sync.dma_start(out=asnd_f[:], in_=asnd_d)
            arec_f = const.tile([U, 1], f32)
            nc.sync.dma_start(out=arec_f[:], in_=arec_d)

            eye_sb = const.tile([P, P], f32)
            nc.sync.dma_start(out=eye_sb[:], in_=eye_d)
            ones_bf = const.tile([1, P], bf16)
            nc.vector.memset(ones_bf[:], 1.0)
            w_r = const.tile([F, U], f32r)
            nc.vector.tensor_copy(w_r[:], w_sb[:])

            # ---------------- x load + xT ----------------
            x_sb = const.tile([P, NT, F], f32)
            x_dma = nc.sync.dma_start(out=x_sb[:],
                                      in_=x_d.rearrange("(t p) f -> p t f", p=P))

            xT_sb = const.tile([F, NT, P], f32r)
            for g in range(4):
                ps = spsum.tile([P, 512], f32, tag="tp")
                for k in range(4):
                    i = 4 * g + k
                    nc.tensor.transpose(ps[:, k * P:(k + 1) * P], x_sb[:, i, :], eye_sb[:])
                nc.scalar.copy(xT_sb.rearrange("f t p -> f (t p)")[:, g * 512:(g + 1) * 512],
                               ps[:])

            # ---------------- W^T, w_s, w_r ----------------
            psw = spsum.tile([P, P], f32, tag="small")
            nc.tensor.transpose(psw[:], w_sb[:], eye_sb[:])
            wT_sb = setup.tile([U, F], f32)
            nc.any.tensor_copy(wT_sb[:], psw[:])

            ps_wsr = spsum.tile([P, 2], f32, tag="small")
            nc.tensor.matmul(ps_wsr[:, 0:1], lhsT=wT_sb[:], rhs=asnd_f[:],
                             start=True, stop=True)
            nc.tensor.matmul(ps_wsr[:, 1:2], lhsT=wT_sb[:], rhs=arec_f[:],
                             start=True, stop=True)
            wsr_sb = setup.tile([F, 2], f32r)
            nc.any.tensor_copy(wsr_sb[:], ps_wsr[:])

            # ---------------- e_s / e_r rows (f32r matmuls, N=512 chunks) -----
            es_row = setup.tile([1, N], bf16)
            er_row = setup.tile([1, N], f32)
            for c in range(4):
                xchunk = xT_sb.rearrange("f t p -> f (t p)")[:, c * 512:(c + 1) * 512]
                ps_es = spsum.tile([1, 512], f32, tag="small")
                nc.tensor.matmul(ps_es[:], lhsT=wsr_sb[:, 0:1], rhs=xchunk,
                                 start=True, stop=True)
                nc.any.tensor_copy(es_row[:, c * 512:(c + 1) * 512], ps_es[:])
                ps_err = spsum.tile([1, 512], f32, tag="small")
                nc.tensor.matmul(ps_err[:], lhsT=wsr_sb[:, 1:2], rhs=xchunk,
                                 start=True, stop=True)
                nc.any.tensor_copy(er_row[:, c * 512:(c + 1) * 512], ps_err[:])

            # e_r as per-partition columns: 16 tiny [1,128]->[128,1] transposes
            ps_er = spsum.tile([P, NT], f32, tag="small")
            for j in range(NT):
                nc.tensor.transpose(ps_er[:, j:j + 1],
                                    er_row[:, j * P:(j + 1) * P],
                                    eye_sb[0:1, 0:1])
            er_sb = const.tile([P, NT], f32)
            nc.any.tensor_copy(er_sb[:], ps_er[:])
            er02_sb = const.tile([P, NT], f32)
            nc.vector.tensor_scalar(er02_sb[:], er_sb[:], ALPHA, None, op0=OP.mult)

            E_sb = const.tile([P, N], bf16)
            for c in range(4):
                ps_E = spsum.tile([P, 512], f32, tag="tp")
                nc.tensor.matmul(ps_E[:], lhsT=ones_bf[:],
                                 rhs=es_row[:, c * 512:(c + 1) * 512],
                                 start=True, stop=True)
                nc.any.tensor_copy(E_sb[:, c * 512:(c + 1) * 512], ps_E[:])

            # ---------------- h (bf16, natural layout) ----------------
            h_sb = const.tile([P, NT, U], bf16)
            for g in range(4):
                psh = spsum.tile([P, 512], f32, tag="tp")
                for k in range(4):
                    i = 4 * g + k
                    nc.tensor.matmul(psh[:, k * P:(k + 1) * P], lhsT=xT_sb[:, i, :],
                                     rhs=w_r[:], start=True, stop=True)
                nc.scalar.copy(h_sb.rearrange("p t u -> p (t u)")[:, g * 512:(g + 1) * 512],
                               psh[:])

            # ---------------- main loop over r-tiles ----------------
            outT_ps = mpsum.tile([U, N], f32)   # 4 PSUM banks, accumulated over j
            n_chunks = NT // DMA_CHUNK
            for g in range(n_chunks):
                adjt_sb = adjp.tile([P, DMA_CHUNK, N], bf16, tag="adjt")
                adj_dma = nc.gpsimd.dma_start(
                    out=adjt_sb[:],
                    in_=adjt_d[g * DMA_CHUNK * P:(g + 1) * DMA_CHUNK * P, :]
                    .rearrange("(c p) s -> p c s", p=P))
                if g == 0:
                    # keep the small setup DMAs ahead of the big adjT stream
                    bass._add_dep_helper(adj_dma.ins, x_dma.ins, sync=True,
                                         reason="x before adjT flood")
                for cc in range(DMA_CHUNK):
                    j = g * DMA_CHUNK + cc
                    a_j = work.tile([P, N], bf16, tag="a")
                    if (j * K_ACT) // NT != ((j + 1) * K_ACT) // NT:
                        # ACT-prelu tile (K_ACT of 16, spread out)
                        nc.scalar.activation(a_j[:], E_sb[:], AF.Prelu,
                                             bias=er_sb[:, j:j + 1], scale=1.0,
                                             alpha=ALPHA)
                    else:
                        z_j = work.tile([P, N], bf16, tag="z")
                        if Z_ENG == "gpsimd":
                            nc.gpsimd.tensor_scalar(z_j[:], E_sb[:],
                                                    er_sb[:, j:j + 1], None,
                                                    op0=OP.add)
                        else:
                            nc.vector.tensor_scalar(z_j[:], E_sb[:],
                                                    er_sb[:, j:j + 1], None,
                                                    op0=OP.add)
                        t_j = work.tile([P, N], bf16, tag="t")
                        nc.vector.tensor_scalar(t_j[:], E_sb[:], ALPHA,
                                                er02_sb[:, j:j + 1],
                                                op0=OP.mult, op1=OP.add)
                        nc.vector.tensor_max(a_j[:], z_j[:], t_j[:])
                    p_j = work.tile([P, N], bf16, tag="p")
                    nc.scalar.activation(p_j[:], a_j[:], AF.Exp)
                    pm_j = work.tile([P, N], bf16, tag="pm")
                    den_j = work.tile([P, 1], f32, tag="den")
                    nc.vector.scalar_tensor_tensor(
                        pm_j[:], p_j[:], 0.0, adjt_sb[:, cc, :],
                        op0=OP.bypass, op1=OP.mult, accum_out=den_j[:])
                    inv_j = work.tile([P, 1], f32, tag="inv")
                    nc.vector.reciprocal(inv_j[:], den_j[:])
                    hp_j = work.tile([P, U], bf16, tag="hp")
                    nc.vector.tensor_scalar(hp_j[:], h_sb[:, j, :], inv_j[:], None,
                                            op0=OP.mult)
                    for c in range(4):
                        nc.tensor.matmul(outT_ps[:, c * 512:(c + 1) * 512],
                                         lhsT=hp_j[:],
                                         rhs=pm_j[:, c * 512:(c + 1) * 512],
                                         start=(j == 0), stop=(j == NT - 1))

            # ---------------- store ----------------
            outT_sb = setup.tile([U, N], f32)
            for c in range(4):
                nc.any.tensor_copy(outT_sb[:, c * 512:(c + 1) * 512],
                                   outT_ps[:, c * 512:(c + 1) * 512])
            nc.sync.dma_start(out=outT_d, in_=outT_sb[:])

    nc.compile()
    return nc


def kernel(x, adj, W_pre, a_snd, a_rec):
    """Full inputs in, full output out. Shards batch across 8 NeuronCores."""
    if "nc" not in _cache:
        _cache["nc"] = _build_nc()
    nc = _cache["nc"]

    x = np.ascontiguousarray(np.asarray(x, dtype=np.float32))
    adj = np.asarray(adj, dtype=np.float32)
    W_pre = np.ascontiguousarray(np.asarray(W_pre, dtype=np.float32))
    a_snd = np.ascontiguousarray(np.asarray(a_snd, dtype=np.float32).reshape(U, 1))
    a_rec = np.ascontiguousarray(np.asarray(a_rec, dtype=np.float32).reshape(U, 1))

    # self-loops (reference: min(1, adj + I)), then receiver-major layout per batch
    adjt = np.ascontiguousarray(adj.transpose(0, 2, 1))
    idx = np.arange(N)
    adjt[:, idx, idx] = 1.0

    eye = np.eye(P, dtype=np.float32)
    in_maps = [
        {"x": x[b], "adjt": adjt[b], "w": W_pre, "asnd": a_snd, "arec": a_rec,
         "eye": eye}
        for b in range(B)
    ]
    trace = bool(int(os.environ.get("GAT_TRACE", "0")))
    res = run_bass_kernel_spmd(nc, in_maps, core_ids=list(range(B)), trace=trace,
                               trace_cores=list(range(B)) if trace else None)
    _cache["last_result"] = res
    out = np.stack([np.ascontiguousarray(r["outT"].T) for r in res.results])
    return out.astype(np.float32)


# revision 17
# speedup vs baseline: 1.7157x; 1.0953x over previous
"""GAT message-passing kernel for Trainium2 (Bass/Tile), 8-core data parallel.

Problem: nn_GAT1 — per batch b:
    h = x @ W_pre                                   [N, U]
    e_s = h @ a_snd ; e_r = h @ a_rec               [N]
    logits[s, r] = leaky_relu(e_s[s] + e_r[r], 0.2)
    att = softmax over senders s (edges only, adj + self-loops)
    out[s, u] = sum_r att[s, r] * h[r, u]

Sharding: data-parallel over batch (B=8 -> one batch per NeuronCore).

Device layout is receiver-major ("transposed", r on partitions, s on free):
  - host passes xT (x transposed) and adjb[r, s] = (adjT - 1) * 1e9, i.e. an
    additive mask: 0 on edges (incl. self-loops), -1e9 on non-edges.
  - adjb is cast fp32->bf16 during the DMA itself (SWDGE cast), so the HBM
    read is still the full fp32 adjacency.
  - logitsT[r, s] = e_s[s] (broadcast matrix E) + e_r[r] (per-partition bias),
    leaky-relu via Prelu on ScalarE or TS/TS/max on VectorE.
  - masked logits = logitsT + adjb  (one 2x tensor_tensor add)
  - pmT = Exp(masked) on ScalarE, whose accum_out gives the softmax
    denominator for free.
  - outT[u, s] = sum_r (h[r, u] / den[r]) * pmT[r, s]  (PE, weight-stationary)
Host transposes outT back when gathering.
"""
import os
import sys

sys.path.insert(0, "/opt/trn_rl_repo")
sys.path.insert(0, "/opt/trn_rl_repo/concourse")

import numpy as np

import concourse.bass as bass
import concourse.bacc as bacc
import concourse.tile as tile
from concourse import mybir
from concourse.bass_utils import run_bass_kernel_spmd

B, N, F, U = 8, 2048, 128, 128
P = 128
NT = N // P          # 16 row tiles
ALPHA = 0.2          # leaky-relu slope
BIG = 1.0e9

K_ACT = int(os.environ.get("GAT_K_ACT", "7"))
DMA_CHUNK = int(os.environ.get("GAT_DMA_CHUNK", "4"))   # r-tiles per adjb DMA
Z_ENG = os.environ.get("GAT_Z_ENG", "gpsimd")           # "vector" | "gpsimd"

f32 = mybir.dt.float32
f32r = mybir.dt.float32r
bf16 = mybir.dt.bfloat16
AF = mybir.ActivationFunctionType
OP = mybir.AluOpType

_cache = {}


def _build_nc():
    nc = bacc.Bacc("TRN2", target_bir_lowering=False, debug=False,
                   enable_asserts=False, num_devices=B)

    xt_d = nc.dram_tensor("xt", [F, N], f32r, kind="ExternalInput").ap()
    adjb_d = nc.dram_tensor("adjb", [N, N], f32, kind="ExternalInput").ap()
    w_d = nc.dram_tensor("w", [F, U], f32, kind="ExternalInput").ap()
    asnd_d = nc.dram_tensor("asnd", [U, 1], f32, kind="ExternalInput").ap()
    arec_d = nc.dram_tensor("arec", [U, 1], f32, kind="ExternalInput").ap()
    eye_d = nc.dram_tensor("eye", [P, P], f32, kind="ExternalInput").ap()
    outT_d = nc.dram_tensor("outT", [U, N], f32, kind="ExternalOutput").ap()

    with tile.TileContext(nc) as tc:
        with (
            tc.tile_pool(name="const", bufs=1) as const,
            tc.tile_pool(name="setup", bufs=2) as setup,
            tc.tile_pool(name="spsum", bufs=2, space="PSUM") as spsum,
            tc.tile_pool(name="adjp", bufs=3) as adjp,
            tc.tile_pool(name="work", bufs=3) as work,
            tc.tile_pool(name="mpsum", bufs=1, space="PSUM") as mpsum,
        ):
            # ---------------- constants ----------------
            w_sb = const.tile([F, U], f32)
            nc.sync.dma_start(out=w_sb[:], in_=w_d)
            asnd_f = const.tile([U, 1], f32)
            nc.sync.dma_start(out=asnd_f[:], in_=asnd_d)
            arec_f = const.tile([U, 1], f32)
            nc.sync.dma_start(out=arec_f[:], in_=arec_d)
            eye_sb = const.tile([P, P], f32)
            nc.sync.dma_start(out=eye_sb[:], in_=eye_d)
            ones_bf = const.tile([1, P], bf16)
            nc.vector.memset(ones_bf[:], 1.0)
            w_r = const.tile([F, U], f32r)
            nc.vector.tensor_copy(w_r[:], w_sb[:])

            # ---------------- xT load (host-transposed) ----------------
            xT_sb = const.tile([F, N], f32r)
            xt_dma = nc.sync.dma_start(out=xT_sb[:], in_=xt_d)
            xT_t = xT_sb.rearrange("f (t p) -> f t p", p=P)

            # ---------------- W^T, w_s, w_r vectors ----------------
            psw = spsum.tile([P, P], f32, tag="small")
            nc.tensor.transpose(psw[:], w_sb[:], eye_sb[:])
            wT_sb = setup.tile([U, F], f32)
            nc.any.tensor_copy(wT_sb[:], psw[:])

            ps_wsr = spsum.tile([P, 2], f32, tag="small")
            nc.tensor.matmul(ps_wsr[:, 0:1], lhsT=wT_sb[:], rhs=asnd_f[:],
                             start=True, stop=True)
            nc.tensor.matmul(ps_wsr[:, 1:2], lhsT=wT_sb[:], rhs=arec_f[:],
                             start=True, stop=True)
            wsr_sb = setup.tile([F, 2], f32r)
            nc.any.tensor_copy(wsr_sb[:], ps_wsr[:])

            # ---------------- e_s / e_r rows (f32r matmuls, N=512 chunks) -----
            es_row = setup.tile([1, N], bf16)
            er_row = setup.tile([1, N], f32)
            for c in range(4):
                xchunk = xT_sb[:, c * 512:(c + 1) * 512]
                ps_es = spsum.tile([1, 512], f32, tag="small")
                nc.tensor.matmul(ps_es[:], lhsT=wsr_sb[:, 0:1], rhs=xchunk,
                                 start=True, stop=True)
                nc.any.tensor_copy(es_row[:, c * 512:(c + 1) * 512], ps_es[:])
                ps_err = spsum.tile([1, 512], f32, tag="small")
                nc.tensor.matmul(ps_err[:], lhsT=wsr_sb[:, 1:2], rhs=xchunk,
                                 start=True, stop=True)
                nc.any.tensor_copy(er_row[:, c * 512:(c + 1) * 512], ps_err[:])

            # e_r as per-partition columns: 16 tiny [1,128]->[128,1] transposes
            ps_er = spsum.tile([P, NT], f32, tag="small")
            for j in range(NT):
                nc.tensor.transpose(ps_er[:, j:j + 1],
                                    er_row[:, j * P:(j + 1) * P],
                                    eye_sb[0:1, 0:1])
            er_sb = const.tile([P, NT], f32)
            nc.any.tensor_copy(er_sb[:], ps_er[:])
            er02_sb = const.tile([P, NT], f32)
            nc.vector.tensor_scalar(er02_sb[:], er_sb[:], ALPHA, None, op0=OP.mult)

            # ---------------- E = broadcast(e_s) (bf16) ----------------
            E_sb = const.tile([P, N], bf16)
            for c in range(4):
                ps_E = spsum.tile([P, 512], f32, tag="tp")
                nc.tensor.matmul(ps_E[:], lhsT=ones_bf[:],
                                 rhs=es_row[:, c * 512:(c + 1) * 512],
                                 start=True, stop=True)
                nc.any.tensor_copy(E_sb[:, c * 512:(c + 1) * 512], ps_E[:])

            # ---------------- h (bf16, natural layout) ----------------
            h_sb = const.tile([P, NT, U], bf16)
            for g in range(4):
                psh = spsum.tile([P, 512], f32, tag="tp")
                for k in range(4):
                    i = 4 * g + k
                    nc.tensor.matmul(psh[:, k * P:(k + 1) * P], lhsT=xT_t[:, i, :],
                                     rhs=w_r[:], start=True, stop=True)
                nc.scalar.copy(h_sb.rearrange("p t u -> p (t u)")[:, g * 512:(g + 1) * 512],
                               psh[:])

            # ---------------- main loop over r-tiles ----------------
            outT_ps = mpsum.tile([U, N], f32)   # 4 PSUM banks, accumulated over j
            n_chunks = NT // DMA_CHUNK
            for g in range(n_chunks):
                adjb_sb = adjp.tile([P, DMA_CHUNK, N], bf16, tag="adjb")
                adj_dma = nc.gpsimd.dma_start(
                    out=adjb_sb[:],
                    in_=adjb_d[g * DMA_CHUNK * P:(g + 1) * DMA_CHUNK * P, :]
                    .rearrange("(c p) s -> p c s", p=P))
                if g == 0:
                    # keep the small setup DMAs ahead of the big adjb stream
                    bass._add_dep_helper(adj_dma.ins, xt_dma.ins, sync=True,
                                         reason="xt before adjb flood")
                for cc in range(DMA_CHUNK):
                    j = g * DMA_CHUNK + cc
                    a_j = work.tile([P, N], bf16, tag="a")
                    if (j * K_ACT) // NT != ((j + 1) * K_ACT) // NT:
                        # ACT-prelu tile (K_ACT of NT, spread out)
                        nc.scalar.activation(a_j[:], E_sb[:], AF.Prelu,
                                             bias=er_sb[:, j:j + 1], scale=1.0,
                                             alpha=ALPHA)
                    else:
                        z_j = work.tile([P, N], bf16, tag="z")
                        if Z_ENG == "gpsimd":
                            nc.gpsimd.tensor_scalar(z_j[:], E_sb[:],
                                                    er_sb[:, j:j + 1], None,
                                                    op0=OP.add)
                        else:
                            nc.vector.tensor_scalar(z_j[:], E_sb[:],
                                                    er_sb[:, j:j + 1], None,
                                                    op0=OP.add)
                        t_j = work.tile([P, N], bf16, tag="t")
                        nc.vector.tensor_scalar(t_j[:], E_sb[:], ALPHA,
                                                er02_sb[:, j:j + 1],
                                                op0=OP.mult, op1=OP.add)
                        nc.vector.tensor_max(a_j[:], z_j[:], t_j[:])
                    am_j = work.tile([P, N], bf16, tag="am")
                    nc.vector.tensor_add(am_j[:], a_j[:], adjb_sb[:, cc, :])
                    pm_j = work.tile([P, N], bf16, tag="pm")
                    den_j = work.tile([P, 1], f32, tag="den")
                    nc.scalar.activation(pm_j[:], am_j[:], AF.Exp,
                                         accum_out=den_j[:])
                    inv_j = work.tile([P, 1], f32, tag="inv")
                    nc.vector.reciprocal(inv_j[:], den_j[:])
                    hp_j = work.tile([P, U], bf16, tag="hp")
                    nc.vector.tensor_scalar(hp_j[:], h_sb[:, j, :], inv_j[:], None,
                                            op0=OP.mult)
                    for c in range(4):
                        nc.tensor.matmul(outT_ps[:, c * 512:(c + 1) * 512],
                                         lhsT=hp_j[:],
                                         rhs=pm_j[:, c * 512:(c + 1) * 512],
                                         start=(j == 0), stop=(j == NT - 1))

            # ---------------- store ----------------
            outT_sb = setup.tile([U, N], f32)
            for c in range(4):
                nc.any.tensor_copy(outT_sb[:, c * 512:(c + 1) * 512],
                                   outT_ps[:, c * 512:(c + 1) * 512])
            nc.sync.dma_start(out=outT_d, in_=outT_sb[:])

    nc.compile()
    return nc


def kernel(x, adj, W_pre, a_snd, a_rec):
    """Full inputs in, full output out. Shards batch across 8 NeuronCores."""
    if "nc" not in _cache:
        _cache["nc"] = _build_nc()
    nc = _cache["nc"]

    x = np.asarray(x, dtype=np.float32)
    adj = np.asarray(adj, dtype=np.float32)
    W_pre = np.ascontiguousarray(np.asarray(W_pre, dtype=np.float32))
    a_snd = np.ascontiguousarray(np.asarray(a_snd, dtype=np.float32).reshape(U, 1))
    a_rec = np.ascontiguousarray(np.asarray(a_rec, dtype=np.float32).reshape(U, 1))

    # receiver-major additive mask: 0 on edges (+self-loops), -1e9 off edges
    adjb = np.ascontiguousarray(adj.transpose(0, 2, 1))
    idx = np.arange(N)
    adjb[:, idx, idx] = 1.0
    adjb -= 1.0
    adjb *= BIG

    xt = np.ascontiguousarray(x.transpose(0, 2, 1))   # [B, F, N]
    eye = np.eye(P, dtype=np.float32)
    in_maps = [
        {"xt": xt[b], "adjb": adjb[b], "w": W_pre, "asnd": a_snd, "arec": a_rec,
         "eye": eye}
        for b in range(B)
    ]
    trace = bool(int(os.environ.get("GAT_TRACE", "0")))
    res = run_bass_kernel_spmd(nc, in_maps, core_ids=list(range(B)), trace=trace,
                               trace_cores=list(range(B)) if trace else None)
    _cache["last_result"] = res
    out = np.stack([np.ascontiguousarray(r["outT"].T) for r in res.results])
    return out.astype(np.float32)
